# revision 8
# baseline (speedup 1.0000x reference)
"""CurveAttention kernel for Trainium2 (8 NeuronCores, head-parallel).

reference:
    scale = D ** -0.5
    S = einsum('bhqd,bhkd->bhqk', Q, K) * scale
    W = exp(S) / (1 + e**e)          # fixed-denominator "softemax"
    O = einsum('bhqk,bhkd->bhqd', W, V)
    returns (O, W)

Sharding: B*H = 64 (batch, head) pairs, 8 per core, no cross-core comms.

Per-core program (per head):
  - load Q,K,V [2048,64]; PE-transpose Q,K -> qT,kT [64,2048] fp32r
  - S-pipe  (store side): per q-block: 4 matmuls -> PSUM [128,2048]
      -> Exp(scale*s - ln denom) -> W row [128,2048] fp32 -> 1 MiB DMA store
  - ST-pipe (AV side): per k-block: matmuls with lhsT/rhs roles swapped
      -> PSUM [128 k, 1024 q] -> Exp -> W^T stripe bf16;
      every 4 k-blocks: AV matmuls (V stationary bf16, W^T moving)
      accumulate in PSUM then DVE-add into out^T [64,2048]
  - out^T -> PE-transpose -> [2048, 64] -> DMA store
"""
import sys

sys.path.insert(0, "/opt/trn_rl_repo")

import numpy as np

B, H, S, D = 4, 16, 2048, 64
N_CORES = 8
HEADS_PER_CORE = (B * H) // N_CORES  # 8

_E = float(np.e)
_DENOM = float(np.exp(0.0) + np.exp(_E))
_LN_DENOM = float(np.log(_DENOM))
_SCALE = float(D) ** -0.5

QB = S // 128  # 16 q-blocks per head
GROUP = 4      # k-blocks accumulated per AV PSUM flush

_cached = None


def _build(n_heads=HEADS_PER_CORE):
    import concourse.bacc as bacc
    import concourse.mybir as mybir
    import concourse.tile as tile
    from concourse.masks import make_identity

    nc = bacc.Bacc("TRN2", target_bir_lowering=False, debug=False,
                   num_devices=N_CORES)

    q_d = nc.dram_tensor("q", [n_heads, S, D], mybir.dt.float32,
                         kind="ExternalInput")
    k_d = nc.dram_tensor("k", [n_heads, S, D], mybir.dt.float32,
                         kind="ExternalInput")
    v_d = nc.dram_tensor("v", [n_heads, S, D], mybir.dt.float32,
                         kind="ExternalInput")
    w_d = nc.dram_tensor("w", [n_heads, S, S], mybir.dt.float32,
                         kind="ExternalOutput")
    o_d = nc.dram_tensor("o", [n_heads, S, D], mybir.dt.float32,
                         kind="ExternalOutput")

    f32 = mybir.dt.float32
    f32r = mybir.dt.float32r
    bf16 = mybir.dt.bfloat16
    EXP = mybir.ActivationFunctionType.Exp

    with tile.TileContext(nc) as tc:
        with (
            tc.tile_pool(name="const", bufs=1) as constp,
            tc.tile_pool(name="io", bufs=2) as iop,
            tc.tile_pool(name="qkt", bufs=2) as qktp,
            tc.tile_pool(name="wrow", bufs=3) as wrowp,
            tc.tile_pool(name="wtst", bufs=6) as wtp,
            tc.tile_pool(name="oacc", bufs=2) as oaccp,
            tc.tile_pool(name="ps_s", bufs=1, space="PSUM") as ps_s,
            tc.tile_pool(name="ps_st", bufs=1, space="PSUM") as ps_st,
            tc.tile_pool(name="ps_sm", bufs=2, space="PSUM") as ps_sm,
        ):
            ident = constp.tile([128, 128], f32)
            make_identity(nc, ident[:])
            bias_t = constp.tile([128, 1], f32)
            nc.gpsimd.memset(bias_t[:], -_LN_DENOM)

            for h in range(n_heads):
                # ---- load + transpose inputs ----
                qn = iop.tile([128, QB, D], f32, tag="qn")
                kn = iop.tile([128, QB, D], f32, tag="kn")
                vn = iop.tile([128, QB, D], f32, tag="vn")
                nc.sync.dma_start(qn[:], q_d[:][h].rearrange("(n p) d -> p n d", p=128))
                nc.sync.dma_start(kn[:], k_d[:][h].rearrange("(n p) d -> p n d", p=128))
                nc.sync.dma_start(vn[:], v_d[:][h].rearrange("(n p) d -> p n d", p=128))

                qT = qktp.tile([64, S], f32r, tag="qT")
                kT = qktp.tile([64, S], f32r, tag="kT")
                vb = qktp.tile([128, QB, D], bf16, tag="vb")
                nc.vector.tensor_copy(vb[:], vn[:])
                for i in range(QB):
                    # small PSUM tiles all share one 1-bank tag ("sm")
                    pt = ps_sm.tile([64, 128], f32, tag="sm")
                    nc.tensor.transpose(pt[:], qn[:, i, :], ident[:])
                    nc.vector.tensor_copy(qT[:, i * 128:(i + 1) * 128], pt[:])
                    pt2 = ps_sm.tile([64, 128], f32, tag="sm")
                    nc.tensor.transpose(pt2[:], kn[:, i, :], ident[:])
                    nc.vector.tensor_copy(kT[:, i * 128:(i + 1) * 128], pt2[:])

                oT = oaccp.tile([64, S], f32, tag="oT")

                # ---- main interleaved loop: S-pipe (store) + ST-pipe (AV) ----
                wts_group = []
                for i in range(QB):
                    # S-pipe: q-block i -> W row [128 q, 2048 k] -> store
                    sps = ps_s.tile([128, S], f32, tag="s")
                    for c in range(4):
                        nc.tensor.matmul(
                            sps[:, c * 512:(c + 1) * 512],
                            qT[:, i * 128:(i + 1) * 128],
                            kT[:, c * 512:(c + 1) * 512],
                            start=True, stop=True,
                        )
                    wrow = wrowp.tile([128, S], f32, tag="wrow")
                    nc.scalar.activation(wrow[:], sps[:], EXP,
                                         bias=bias_t[:], scale=_SCALE)
                    nc.sync.dma_start(w_d[:][h, i * 128:(i + 1) * 128, :],
                                      wrow[:])

                    # ST-pipe: k-block i -> W^T stripe bf16
                    kb = i
                    wts = wtp.tile([128, S], bf16, tag="wts")
                    for half in range(2):
                        stps = ps_st.tile([128, 1024], f32, tag="st")
                        for c in range(2):
                            nc.tensor.matmul(
                                stps[:, c * 512:(c + 1) * 512],
                                kT[:, kb * 128:(kb + 1) * 128],
                                qT[:, half * 1024 + c * 512:
                                   half * 1024 + (c + 1) * 512],
                                start=True, stop=True,
                            )
                        nc.scalar.activation(
                            wts[:, half * 1024:(half + 1) * 1024], stps[:],
                            EXP, bias=bias_t[:], scale=_SCALE)
                    wts_group.append((kb, wts))

                    # AV sweep every GROUP k-blocks:
                    # out^T[:, qc] += sum_{kb in group} V[kb].T-contract W^T
                    if len(wts_group) == GROUP:
                        first_group = wts_group[0][0] == 0
                        for qc in range(4):
                            avp = ps_sm.tile([64, 512], f32, tag="sm")
                            for j, (kbj, wtsj) in enumerate(wts_group):
                                nc.tensor.matmul(
                                    avp[:],
                                    vb[:, kbj, :],
                                    wtsj[:, qc * 512:(qc + 1) * 512],
                                    start=(j == 0), stop=(j == GROUP - 1),
                                )
                            dst = oT[:, qc * 512:(qc + 1) * 512]
                            if first_group:
                                nc.vector.tensor_copy(dst, avp[:])
                            else:
                                nc.vector.tensor_add(dst, dst, avp[:])
                        wts_group = []

                # ---- out^T -> out, store ----
                ost = iop.tile([128, QB, D], f32, tag="ost")
                for i in range(QB):
                    pt3 = ps_sm.tile([128, 64], f32, tag="sm")
                    # transpose [64,128] -> [128,64]; rhs identity is [64,64]
                    nc.tensor.transpose(pt3[:],
                                        oT[:, i * 128:(i + 1) * 128],
                                        ident[:64, :64])
                    nc.vector.tensor_copy(ost[:, i, :], pt3[:])
                nc.sync.dma_start(o_d[:][h].rearrange("(n p) d -> p n d", p=128),
                                  ost[:])

    nc.compile()
    return nc


def _get_nc():
    global _cached
    if _cached is None:
        _cached = _build()
    return _cached


def kernel(query, key, value):
    from concourse.bass_utils import run_bass_kernel_spmd

    nc = _get_nc()

    q = np.ascontiguousarray(np.asarray(query, dtype=np.float32)
                             .reshape(B * H, S, D))
    k = np.ascontiguousarray(np.asarray(key, dtype=np.float32)
                             .reshape(B * H, S, D))
    v = np.ascontiguousarray(np.asarray(value, dtype=np.float32)
                             .reshape(B * H, S, D))

    in_maps = []
    for c in range(N_CORES):
        sl = slice(c * HEADS_PER_CORE, (c + 1) * HEADS_PER_CORE)
        in_maps.append({
            "q": np.ascontiguousarray(q[sl]),
            "k": np.ascontiguousarray(k[sl]),
            "v": np.ascontiguousarray(v[sl]),
        })

    res = run_bass_kernel_spmd(nc, in_maps, core_ids=list(range(N_CORES)))

    attn_w = np.empty((B * H, S, S), dtype=np.float32)
    attn_o = np.empty((B * H, S, D), dtype=np.float32)
    for c in range(N_CORES):
        sl = slice(c * HEADS_PER_CORE, (c + 1) * HEADS_PER_CORE)
        attn_w[sl] = res.results[c]["w"]
        attn_o[sl] = res.results[c]["o"]

    return (attn_o.reshape(B, H, S, D), attn_w.reshape(B, H, S, S))


# revision 9
# speedup vs baseline: 1.2003x; 1.2003x over previous
"""CurveAttention kernel for Trainium2 (8 NeuronCores, head-parallel).

reference:
    scale = D ** -0.5
    S = einsum('bhqd,bhkd->bhqk', Q, K) * scale
    W = exp(S) / (1 + e**e)          # fixed-denominator "softemax"
    O = einsum('bhqk,bhkd->bhqd', W, V)
    returns (O, W)

Sharding: B*H = 64 (batch, head) pairs, 8 per core, no cross-core comms.

Per-core program (per head):
  - load Q,K,V [2048,64]; PE-transpose Q,K -> qT,kT [64,2048] fp32r
  - S-pipe  (store side): per q-block: 4 matmuls -> PSUM [128,2048]
      -> Exp(scale*s - ln denom) -> W row [128,2048] fp32 -> 1 MiB DMA store
  - ST-pipe (AV side): per k-block: matmuls with lhsT/rhs roles swapped
      -> PSUM [128 k, 1024 q] -> Exp -> W^T stripe bf16;
      every 4 k-blocks: AV matmuls (V stationary bf16, W^T moving)
      accumulate in PSUM then DVE-add into out^T [64,2048]
  - out^T -> PE-transpose -> [2048, 64] -> DMA store
"""
import sys

sys.path.insert(0, "/opt/trn_rl_repo")

import numpy as np

B, H, S, D = 4, 16, 2048, 64
N_CORES = 8
HEADS_PER_CORE = (B * H) // N_CORES  # 8

_E = float(np.e)
_DENOM = float(np.exp(0.0) + np.exp(_E))
_LN_DENOM = float(np.log(_DENOM))
_SCALE = float(D) ** -0.5

QB = S // 128  # 16 q-blocks per head
GROUP = 4      # k-blocks accumulated per AV PSUM flush

_cached = None


def _build(n_heads=HEADS_PER_CORE):
    import concourse.bacc as bacc
    import concourse.mybir as mybir
    import concourse.tile as tile
    from concourse.masks import make_identity

    nc = bacc.Bacc("TRN2", target_bir_lowering=False, debug=False,
                   num_devices=N_CORES)

    q_d = nc.dram_tensor("q", [n_heads, S, D], mybir.dt.float32,
                         kind="ExternalInput")
    k_d = nc.dram_tensor("k", [n_heads, S, D], mybir.dt.float32,
                         kind="ExternalInput")
    v_d = nc.dram_tensor("v", [n_heads, S, D], mybir.dt.float32,
                         kind="ExternalInput")
    w_d = nc.dram_tensor("w", [n_heads, S, S], mybir.dt.float32,
                         kind="ExternalOutput")
    o_d = nc.dram_tensor("o", [n_heads, S, D], mybir.dt.float32,
                         kind="ExternalOutput")

    f32 = mybir.dt.float32
    f32r = mybir.dt.float32r
    bf16 = mybir.dt.bfloat16
    EXP = mybir.ActivationFunctionType.Exp

    with tile.TileContext(nc) as tc:
        with (
            tc.tile_pool(name="const", bufs=1) as constp,
            tc.tile_pool(name="io", bufs=2) as iop,
            tc.tile_pool(name="qkt", bufs=2) as qktp,
            tc.tile_pool(name="wrow", bufs=6) as wrowp,
            tc.tile_pool(name="wtst", bufs=6) as wtp,
            tc.tile_pool(name="oacc", bufs=2) as oaccp,
            tc.tile_pool(name="ps_s", bufs=1, space="PSUM") as ps_s,
            tc.tile_pool(name="ps_st", bufs=1, space="PSUM") as ps_st,
            tc.tile_pool(name="ps_sm", bufs=2, space="PSUM") as ps_sm,
        ):
            ident = constp.tile([128, 128], f32)
            make_identity(nc, ident[:])
            bias_t = constp.tile([128, 1], f32)
            nc.gpsimd.memset(bias_t[:], -_LN_DENOM)

            for h in range(n_heads):
                # ---- load + transpose inputs ----
                qn = iop.tile([128, QB, D], f32, tag="qn")
                kn = iop.tile([128, QB, D], f32, tag="kn")
                vn = iop.tile([128, QB, D], f32, tag="vn")
                nc.sync.dma_start(qn[:], q_d[:][h].rearrange("(n p) d -> p n d", p=128))
                nc.sync.dma_start(kn[:], k_d[:][h].rearrange("(n p) d -> p n d", p=128))
                nc.sync.dma_start(vn[:], v_d[:][h].rearrange("(n p) d -> p n d", p=128))

                qT = qktp.tile([64, S], f32r, tag="qT")
                kT = qktp.tile([64, S], f32r, tag="kT")
                vb = qktp.tile([128, QB, D], bf16, tag="vb")
                nc.vector.tensor_copy(vb[:], vn[:])
                for i in range(QB):
                    # small PSUM tiles all share one 1-bank tag ("sm")
                    pt = ps_sm.tile([64, 128], f32, tag="sm")
                    nc.tensor.transpose(pt[:], qn[:, i, :], ident[:])
                    nc.vector.tensor_copy(qT[:, i * 128:(i + 1) * 128], pt[:])
                    pt2 = ps_sm.tile([64, 128], f32, tag="sm")
                    nc.tensor.transpose(pt2[:], kn[:, i, :], ident[:])
                    nc.vector.tensor_copy(kT[:, i * 128:(i + 1) * 128], pt2[:])

                oT = oaccp.tile([64, S], f32, tag="oT")

                # ---- main interleaved loop: S-pipe (store) + ST-pipe (AV) ----
                wts_group = []
                for i in range(QB):
                    # S-pipe: q-block i -> W row [128 q, 2048 k] -> store
                    sps = ps_s.tile([128, S], f32, tag="s")
                    for c in range(4):
                        nc.tensor.matmul(
                            sps[:, c * 512:(c + 1) * 512],
                            qT[:, i * 128:(i + 1) * 128],
                            kT[:, c * 512:(c + 1) * 512],
                            start=True, stop=True,
                        )
                    wrow = wrowp.tile([128, S], f32, tag="wrow")
                    nc.scalar.activation(wrow[:], sps[:], EXP,
                                         bias=bias_t[:], scale=_SCALE)
                    nc.sync.dma_start(w_d[:][h, i * 128:(i + 1) * 128, :],
                                      wrow[:])

                    # ST-pipe: k-block i -> W^T stripe bf16
                    kb = i
                    wts = wtp.tile([128, S], bf16, tag="wts")
                    for half in range(2):
                        stps = ps_st.tile([128, 1024], f32, tag="st")
                        for c in range(2):
                            nc.tensor.matmul(
                                stps[:, c * 512:(c + 1) * 512],
                                kT[:, kb * 128:(kb + 1) * 128],
                                qT[:, half * 1024 + c * 512:
                                   half * 1024 + (c + 1) * 512],
                                start=True, stop=True,
                            )
                        nc.scalar.activation(
                            wts[:, half * 1024:(half + 1) * 1024], stps[:],
                            EXP, bias=bias_t[:], scale=_SCALE)
                    wts_group.append((kb, wts))

                    # AV sweep every GROUP k-blocks:
                    # out^T[:, qc] += sum_{kb in group} V[kb].T-contract W^T
                    if len(wts_group) == GROUP:
                        first_group = wts_group[0][0] == 0
                        for qc in range(4):
                            avp = ps_sm.tile([64, 512], f32, tag="sm")
                            for j, (kbj, wtsj) in enumerate(wts_group):
                                nc.tensor.matmul(
                                    avp[:],
                                    vb[:, kbj, :],
                                    wtsj[:, qc * 512:(qc + 1) * 512],
                                    start=(j == 0), stop=(j == GROUP - 1),
                                )
                            dst = oT[:, qc * 512:(qc + 1) * 512]
                            if first_group:
                                nc.vector.tensor_copy(dst, avp[:])
                            else:
                                nc.vector.tensor_add(dst, dst, avp[:])
                        wts_group = []

                # ---- out^T -> out, store ----
                ost = iop.tile([128, QB, D], f32, tag="ost")
                for i in range(QB):
                    pt3 = ps_sm.tile([128, 64], f32, tag="sm")
                    # transpose [64,128] -> [128,64]; rhs identity is [64,64]
                    nc.tensor.transpose(pt3[:],
                                        oT[:, i * 128:(i + 1) * 128],
                                        ident[:64, :64])
                    nc.vector.tensor_copy(ost[:, i, :], pt3[:])
                nc.sync.dma_start(o_d[:][h].rearrange("(n p) d -> p n d", p=128),
                                  ost[:])

    nc.compile()
    return nc


def _get_nc():
    global _cached
    if _cached is None:
        _cached = _build()
    return _cached


def kernel(query, key, value):
    from concourse.bass_utils import run_bass_kernel_spmd

    nc = _get_nc()

    q = np.ascontiguousarray(np.asarray(query, dtype=np.float32)
                             .reshape(B * H, S, D))
    k = np.ascontiguousarray(np.asarray(key, dtype=np.float32)
                             .reshape(B * H, S, D))
    v = np.ascontiguousarray(np.asarray(value, dtype=np.float32)
                             .reshape(B * H, S, D))

    in_maps = []
    for c in range(N_CORES):
        sl = slice(c * HEADS_PER_CORE, (c + 1) * HEADS_PER_CORE)
        in_maps.append({
            "q": np.ascontiguousarray(q[sl]),
            "k": np.ascontiguousarray(k[sl]),
            "v": np.ascontiguousarray(v[sl]),
        })

    res = run_bass_kernel_spmd(nc, in_maps, core_ids=list(range(N_CORES)))

    attn_w = np.empty((B * H, S, S), dtype=np.float32)
    attn_o = np.empty((B * H, S, D), dtype=np.float32)
    for c in range(N_CORES):
        sl = slice(c * HEADS_PER_CORE, (c + 1) * HEADS_PER_CORE)
        attn_w[sl] = res.results[c]["w"]
        attn_o[sl] = res.results[c]["o"]

    return (attn_o.reshape(B, H, S, D), attn_w.reshape(B, H, S, S))
